# revision 5
# baseline (speedup 1.0000x reference)
"""Multi-head attention (B=8, N=1024, C=1024, H=16) on 8 TRN2 NeuronCores.

Data-parallel over batch: core b computes batch element b end-to-end; no
collectives. All matmuls run in bf16 with fp32 PSUM accumulation, and ALL of
them use the full 128x128 array mode (no tile_position packing) so the PE
never pays a tiling-mode-switch drain regardless of scheduler interleaving.

Per-head-pair trickery (pair p = heads 2p, 2p+1 share a 128-partition tile;
head A on partitions 0:64, head B on 64:128):

  scores  sT_h[j,i]: lhsT = kT_pair [d(128), j(128)] (both heads), rhs = qTz_h
          where qTz_A has q_A rows at 0:64 and ZEROS at 64:128 (mirrored for
          B) -> the zero rows annihilate the other head's k columns, so a
          full-mode K=128 matmul yields exactly one head's scores.
  exp     batched over 2 PSUM banks per ACTIVATE (amortizes the 352-cycle
          fixed cost), writes bf16 pT straight to SBUF in AV-ready layout.
  AV+Z    lhsT_A = [v_A | ones] [j, 128]: out rows 0:64 = attn-weighted v,
          rows 64:128 = Z (softmax denominator) REPLICATED over 64
          partitions -- the otherwise-idle half of the array computes the
          denominator and its cross-partition broadcast for free.
          lhsT_B = [ones | v_B] (ones block shared in a 192-wide
          [v_A | ones | v_B] layout).
  norm    rz = 1/Z via cross-partition-window reciprocal, then
          one tensor_mul per head fused with the bf16 downcast into aT.
  proj    y[n,o]: lhsT = aT tile, rhs = proj_wT; bias-add fused with the
          PSUM drain.
"""

import numpy as np
import ml_dtypes

import concourse.bass as bass
import concourse.tile as tile
import concourse.tile_utils as tile_utils
from concourse import bacc, mybir, bass_utils

tile_utils.max_sbuf_usage = 208 * 1024  # stale 192KiB cap; cayman has 208 usable

N = 1024   # sequence length
C = 1024   # model dim
H = 16     # heads
D = 64     # head dim
CT = 8     # 128-row tiles of c (contraction dim)
NT = 8     # 128-row tiles of n
NB = 2     # 512-wide blocks of n
PAIRS = 8

BF16 = mybir.dt.bfloat16
F32 = mybir.dt.float32

_nc_cache = None


def build_nc():
    global _nc_cache
    if _nc_cache is not None:
        return _nc_cache

    nc = bacc.Bacc("TRN2", target_bir_lowering=False, debug=False, num_devices=8)

    x_d = nc.dram_tensor("x", [C, N], BF16, kind="ExternalInput").ap()
    qkv_w_d = nc.dram_tensor("qkv_w", [C, 3 * C], BF16, kind="ExternalInput").ap()
    proj_w_d = nc.dram_tensor("proj_w", [C, C], BF16, kind="ExternalInput").ap()
    proj_b_d = nc.dram_tensor("proj_b", [C], F32, kind="ExternalInput").ap()
    out_d = nc.dram_tensor("out", [N, C], F32, kind="ExternalOutput").ap()

    Exp = mybir.ActivationFunctionType.Exp
    Copy = mybir.ActivationFunctionType.Copy

    with tile.TileContext(nc) as tc:
        with tc.tile_pool(name="big", bufs=1) as big, \
             tc.tile_pool(name="wk", bufs=2) as wk, \
             tc.tile_pool(name="ps", bufs=2, space="PSUM") as ps:

            # x / qkv weights in 2-c-tile chunks so matmuls start after the
            # first chunk's DMA instead of the whole 8MB load
            xT_s = [big.tile([128, 2, N], BF16, name=f"xT{i}", tag=f"x{i}")
                    for i in range(4)]
            qkv_wT_s = [big.tile([128, 2, 3 * C], BF16, name=f"qw{i}", tag=f"qw{i}")
                        for i in range(4)]
            proj_wT_s = big.tile([128, CT, C], BF16)
            # zero-padded q (per head half), natural k
            qTz_s = big.tile([128, 2, PAIRS, N], BF16)
            kT_s = big.tile([128, PAIRS, N], BF16)
            # [v_A | ones | v_B] per (n-tile, pair): A window 0:128, B 64:192
            von_s = big.tile([128, NT, PAIRS, 3, 64], BF16)
            aT_s = big.tile([128, CT, N], BF16)
            bias_s = big.tile([128, C], F32)

            def xT(ct):
                return xT_s[ct // 2][:, ct % 2, :]

            def qw(ct):
                return qkv_wT_s[ct // 2][:, ct % 2, :]

            # one-time constant fills (gpsimd: keeps DVE/ACT free)
            nc.gpsimd.memset(qTz_s[64:128, 0, :, :], 0.0)
            nc.gpsimd.memset(qTz_s[0:64, 1, :, :], 0.0)
            nc.gpsimd.memset(von_s[:, :, :, 1, :], 1.0)

            for i in range(4):
                nc.sync.dma_start(
                    out=qkv_wT_s[i][:, 0, :], in_=qkv_w_d[i * 256:i * 256 + 128, :])
                nc.sync.dma_start(
                    out=qkv_wT_s[i][:, 1, :], in_=qkv_w_d[i * 256 + 128:(i + 1) * 256, :])
                nc.sync.dma_start(
                    out=xT_s[i][:, 0, :], in_=x_d[i * 256:i * 256 + 128, :])
                nc.sync.dma_start(
                    out=xT_s[i][:, 1, :], in_=x_d[i * 256 + 128:(i + 1) * 256, :])
            bias_bcast = bass.AP(
                tensor=proj_b_d.tensor,
                offset=proj_b_d.offset,
                ap=[[0, 128], proj_b_d.ap[0]],
            )
            nc.gpsimd.dma_start(out=bias_s, in_=bias_bcast)

            def qkv_qk(p):
                for which, ot in ((0, p), (1, 8 + p)):  # 0 = q-tile, 1 = k-tile
                    for nb in range(NB):
                        nbs = slice(nb * 512, (nb + 1) * 512)
                        acc = ps.tile([128, 512], F32, tag="qp", name=f"qk{ot}_{nb}")
                        for ct in range(CT):
                            nc.tensor.matmul(
                                acc,
                                qw(ct)[:, ot * 128:(ot + 1) * 128],
                                xT(ct)[:, nbs],
                                start=(ct == 0), stop=(ct == CT - 1))
                        if which == 0:
                            nc.vector.tensor_copy(
                                out=qTz_s[0:64, 0, p, nbs], in_=acc[0:64, :])
                            nc.vector.tensor_copy(
                                out=qTz_s[64:128, 1, p, nbs], in_=acc[64:128, :])
                        else:
                            # k-cast on ScalarE: DVE is the second-busiest engine
                            nc.scalar.activation(
                                out=kT_s[:, p, nbs], in_=acc, func=Copy)

            def qkv_v(g):
                # v natural layout [n, o'], o'-block g covers pairs 4g..4g+3
                for nt in range(NT):
                    acc = ps.tile([128, 512], F32, tag="qp", name=f"v{nt}_{g}")
                    for ct in range(CT):
                        nc.tensor.matmul(
                            acc,
                            xT(ct)[:, nt * 128:(nt + 1) * 128],
                            qw(ct)[:, 2 * C + g * 512: 2 * C + (g + 1) * 512],
                            start=(ct == 0), stop=(ct == CT - 1))
                    # batched strided copies: all 4 pairs' A-halves, then B-halves
                    accv = acc.rearrange("p (q w e) -> p q w e", q=4, w=2)
                    nc.vector.tensor_copy(
                        out=von_s[:, nt, 4 * g:4 * g + 4, 0, :], in_=accv[:, :, 0, :])
                    nc.vector.tensor_copy(
                        out=von_s[:, nt, 4 * g:4 * g + 4, 2, :], in_=accv[:, :, 1, :])

            def attention(p):
                for ib in range(NB):
                    ibs = slice(ib * 512, (ib + 1) * 512)
                    pT = [wk.tile([128, 8, 512], BF16, tag="pT", bufs=3,
                                  name=f"pT{p}_{ib}_{h}") for h in range(2)]
                    for h in range(2):
                        for jb in range(4):  # 2 j-tiles per psum batch
                            s2 = ps.tile([128, 2, 512], F32, tag="s",
                                         name=f"s{p}_{ib}_{h}_{jb}")
                            for u in range(2):
                                jt = 2 * jb + u
                                nc.tensor.matmul(
                                    s2[:, u, :],
                                    kT_s[:, p, jt * 128:(jt + 1) * 128],
                                    qTz_s[:, h, p, ibs],
                                    start=True, stop=True)
                            nc.scalar.activation(
                                out=pT[h][:, 2 * jb:2 * jb + 2, :], in_=s2,
                                func=Exp, scale=0.125)
                    psA = ps.tile([128, 512], F32, tag="o", name=f"psA{p}_{ib}")
                    psB = ps.tile([128, 512], F32, tag="o", name=f"psB{p}_{ib}")
                    for jt in range(8):
                        nc.tensor.matmul(
                            psA, von_s[:, jt, p, 0:2, :].rearrange("p a b -> p (a b)"),
                            pT[0][:, jt, :],
                            start=(jt == 0), stop=(jt == 7), skip_group_check=True)
                    for jt in range(8):
                        nc.tensor.matmul(
                            psB, von_s[:, jt, p, 1:3, :].rearrange("p a b -> p (a b)"),
                            pT[1][:, jt, :],
                            start=(jt == 0), stop=(jt == 7), skip_group_check=True)
                    # psA rows 64:128 = Z_A replicated; psB rows 0:64 = Z_B
                    rz = wk.tile([128, 512], F32, tag="rz", name=f"rz{p}_{ib}")
                    nc.vector.reciprocal(out=rz[0:64, :], in_=psA[64:128, :])
                    nc.vector.reciprocal(out=rz[64:128, :], in_=psB[0:64, :])
                    nc.vector.tensor_mul(
                        out=aT_s[0:64, p, ibs], in0=psA[0:64, :], in1=rz[0:64, :])
                    nc.vector.tensor_mul(
                        out=aT_s[64:128, p, ibs], in0=psB[64:128, :],
                        in1=rz[64:128, :])

            for g in range(2):
                for p in range(4 * g, 4 * g + 4):
                    qkv_qk(p)
                qkv_v(g)
                for p in range(4 * g, 4 * g + 4):
                    attention(p)

            for ct in range(CT):
                nc.sync.dma_start(
                    out=proj_wT_s[:, ct, :], in_=proj_w_d[ct * 128:(ct + 1) * 128, :])

            for nt in range(NT):
                y = wk.tile([128, C], F32, tag="y", bufs=1, name=f"y{nt}")
                for ob in range(NB):
                    obs = slice(ob * 512, (ob + 1) * 512)
                    acc = ps.tile([128, 512], F32, tag="qp", name=f"pr{nt}_{ob}")
                    for ct in range(CT):
                        nc.tensor.matmul(
                            acc,
                            aT_s[:, ct, nt * 128:(nt + 1) * 128],
                            proj_wT_s[:, ct, obs],
                            start=(ct == 0), stop=(ct == CT - 1))
                    nc.vector.tensor_add(out=y[:, obs], in0=acc, in1=bias_s[:, obs])
                nc.sync.dma_start(out=out_d[nt * 128:(nt + 1) * 128, :], in_=y)

    nc.finalize()
    _nc_cache = nc
    return nc


def kernel(x, qkv_w, proj_w, proj_b, trace=False):
    nc = build_nc()
    bf = ml_dtypes.bfloat16
    x = np.asarray(x, dtype=np.float32)
    qkv_wT = np.ascontiguousarray(np.asarray(qkv_w, dtype=np.float32).T).astype(bf)
    proj_wT = np.ascontiguousarray(np.asarray(proj_w, dtype=np.float32).T).astype(bf)
    proj_b = np.ascontiguousarray(np.asarray(proj_b, dtype=np.float32))

    in_maps = []
    for b in range(8):
        in_maps.append({
            "x": np.ascontiguousarray(x[b].T).astype(bf),
            "qkv_w": qkv_wT,
            "proj_w": proj_wT,
            "proj_b": proj_b,
        })

    res = bass_utils.run_bass_kernel_spmd(
        nc, in_maps, core_ids=list(range(8)), trace=trace)
    out = np.stack([
        np.asarray(res.results[b]["out"], dtype=np.float32) for b in range(8)])
    if trace:
        return out, res
    return out
